# revision 33
# baseline (speedup 1.0000x reference)
"""Trainium2 Bass kernel for the modified-MDPN dendrite model (v7).

Math per output element (b, i, j, m):
    acc = sum_r log(prod_c u)  with  u = atan(10*(x*w - q))/pi + 1.1
        = log(prod_{r,c} u)    (u > 0 always)
Then 4x4 spatial maxpool, flatten (io, jo, m), fc1(7744->128)+relu,
fc2(128->10).

Device strategy (8 NeuronCores, data parallel over batch, 2 images/core):
  - partitions p = 4*(b*16 + m) + cp (b: 2 images, m: 16 filters, cp: 4
    chunks of 22 output rows; 4*22 = 88 = S exactly, so the atan stream
    carries ZERO padding: free size per tap is 22 rows * 88 cols = 1936
    vs the 2112 of a pool-aligned 12-row chunking).
  - per tap (r, c): one ACT Arctan over [128, 1936] with per-partition
    scale=10w, bias=-10q folded into the activation pre-affine (HW arctan
    is accurate far beyond +-pi/2); one DVE tensor_scalar (u = t/pi + 1.1,
    bf16, 4x mode); one DVE tensor_tensor multiply into a running product
    (bf16, 2x mode, ping-pong).  Cost model: ACT 1798ns/tap (1613 compute
    + 185 SBUF access), DVE 1634ns/tap -- ACT is the critical path and
    runs gapless.  Taps run r-major so the input DMA (x pre-cast to bf16
    on host, row slices in order) stays ahead; tap 0 is split into 3
    row-pieces matched to the first DMA slices.  A dependency-free dummy
    atan at the top pulls the 1283ns Arctan table load to t=0 (it is
    otherwise inserted before the first real atan and inherits its DMA
    waits).
  - ln is monotonic, so the 4x4 maxpool runs on the bf16 *products*:
    col-pool (groups of 4 j) is a free-dim tensor_reduce; the row pool
    crosses the 22-row chunks, so the tiny j-pooled map ([128, 24*22]
    bf16, 2 of 24 jo slots zero-pad) is repacked onto partitions
    p' = 4*(b*16+m) + joc, free (cp, jos, il) -- that free order keeps
    both DMA access patterns mergeable to <= 3 dims, allowing ONE
    SBUF->SBUF DMA per joc block (4 total; each costs ~650ns serialized
    HWDGE time, which is why cp=4 beats cp=8).  The last tap runs in 4
    joc-aligned jo-quarters so each quarter's j-pool launches its repack
    DMA immediately, overlapping the remaining quarters' compute; tap 79
    runs in j-halves to shrink the ACT->DVE skew into the tail.
  - After the repack the row pool is slot-based in free dims: chunk cp
    owns row-groups io = IO0[cp] + s; the two groups crossing a chunk
    boundary are finished with a tiny max against the next chunk's first
    two rows (the next chunk is a free-dim neighbour after the repack).
  - Ln never runs on ACT (avoids Arctan<->Ln table reloads, 1283ns each):
    ln(v) ~= ln2*(bits(v)/128 - 127 + 0.043) via a bf16->int16 bitcast +
    one DVE tensor_scalar, folded into the scatter that builds fc1's rhs.
    ~1e-3 additional relative error on the logits.
  - fc1: contraction over (m, jo, io) runs as 132 accumulating K=128
    matmuls (12 all-zero garbage groups skipped); batch lives on
    partitions, so the rhs is a host-zeroed two-column copy of y2ln
    (col b nonzero only on image-b partitions), giving both images'
    hidden vectors in one PSUM [128, 2].  relu+bias and the fc2 bias run
    on DVE (no ACT table traffic); fc2 is one matmul.

Cost model whole-kernel: 163.1us (v1 12-row-chunk baseline: 175.8us on
the same model).  Breakdown: ~3.9us input-DMA-latency startup, 81-tap
ACT stream ~146us, ~12us tail (DVE pool drain 3.4, last repack DMA
chain 2.8, pool/fc chain 2.9, out-DMA + exit barrier 3.0).
"""

import math
import sys

sys.path.insert(0, "/opt/trn_rl_repo")

import ml_dtypes
import numpy as np

import concourse.bacc as bacc
import concourse.mybir as mybir
from concourse import tile
from concourse.bass_utils import run_bass_kernel_spmd

AFT = mybir.ActivationFunctionType
ALU = mybir.AluOpType
F32 = mybir.dt.float32
BF16 = mybir.dt.bfloat16
I16 = mybir.dt.int16

M = 16          # filters
N = 9           # window side
IMG = 96
S = 88          # sliding-window output side
SP = 22         # pooled side
B = 16          # global batch
NCORES = 8
BL = B // NCORES          # images per core (2)
CP = 4                    # row chunks (22 rows each; 4*22 = 88, no pad)
RP = 22                   # output rows per chunk
HALO = RP + N - 1         # input rows per chunk (30)
FD = RP * S               # free elems per tap instruction (1936)
JOP = 24                  # jo slots incl 2 zero-pads (4 joc * 6 jos)
G1 = 4 * 6 * 6            # fc1 groups (cp, jos, slot) incl garbage slots
IO0 = (0, 6, 11, 17)      # first row-group owned by chunk cp
OFF = (0, 2, 0, 2)        # in-chunk il offset of the first owned group
PI = float(np.pi)
# ln(v) ~= LN_S0 * int16_bits(bf16 v) + LN_S1  (positive normal v)
LN_S0 = math.log(2.0) / 128.0
LN_S1 = math.log(2.0) * (0.043 - 127.0)

_CACHE = {}


def _build_nc():
    nc = bacc.Bacc("TRN2", target_bir_lowering=False, debug=False)

    xp = nc.declare_dram_parameter("xp", [128, HALO * IMG], BF16, isOutput=False)
    wq = nc.declare_dram_parameter("wq", [128, 162], F32, isOutput=False)
    w1 = nc.declare_dram_parameter("w1", [128, G1 * 128], BF16, isOutput=False)
    w2b = nc.declare_dram_parameter("w2b", [128, 11], F32, isOutput=False)
    b2 = nc.declare_dram_parameter("b2", [10, 1], F32, isOutput=False)
    out = nc.declare_dram_parameter("out", [10, BL], F32, isOutput=True)

    with tile.TileContext(nc) as tc:
        with (
            tc.tile_pool(name="consts", bufs=1) as cpool,
            tc.tile_pool(name="work", bufs=4) as wpool,
            tc.tile_pool(name="state", bufs=1) as spool,
            tc.tile_pool(name="psum", bufs=1, space="PSUM") as ppool,
        ):
            xs = cpool.tile([128, HALO * IMG], BF16, tag="xs")
            wqt = cpool.tile([128, 162], F32, tag="wqt")
            w1t = cpool.tile([128, G1 * 128], BF16, tag="w1t")
            w2bt = cpool.tile([128, 11], F32, tag="w2bt")
            b2t = cpool.tile([10, 1], F32, tag="b2t")
            wst = wqt[:, 0:81]
            qst = wqt[:, 81:162]

            xsr = xs[:].rearrange("p (il j) -> p il j", il=HALO, j=IMG)
            xpr = xp.rearrange("p (il j) -> p il j", il=HALO, j=IMG)

            # Table-load warmup: the Arctan table load is inserted right
            # before the first ACT instruction and inherits its DMA waits,
            # so give ACT a dependency-free dummy atan first -- the 1283ns
            # load then runs at t=0 under the input-DMA latency.  All DMAs
            # stay on the SP queue (an ACT-queue DMA triggers an extra
            # set-0 table load).  First x slice leads, then ws/qs, the
            # remaining slices, and the big fc1 weights last.
            scr = cpool.tile([128, 1], F32, tag="scr")
            nc.vector.memset(scr[:], 0.0)
            nc.scalar.activation(scr[:], scr[:], AFT.Arctan)
            head = [(0, 6), (6, 14), (14, RP)]
            nc.sync.dma_start(xsr[:, 0:6], xpr[:, 0:6])
            nc.sync.dma_start(wqt[:], wq[:])
            for il0, il1 in head[1:]:
                nc.sync.dma_start(xsr[:, il0:il1], xpr[:, il0:il1])
            nc.sync.dma_start(xsr[:, RP:HALO], xpr[:, RP:HALO])
            nc.sync.dma_start(b2t[:], b2[:])
            nc.sync.dma_start(w2bt[:], w2b[:])
            nc.sync.dma_start(w1t[:], w1[:])

            # j-pooled map, layout (jo, il); jo slots 22..23 stay zero.
            p1 = spool.tile([128, JOP * RP], BF16, tag="p1")
            nc.vector.memset(p1[:], 0.0)
            p1v = p1[:].rearrange("p (jo il) -> p il jo", jo=JOP, il=RP)
            # image-masked two-column rhs for fc1 (memset covers the
            # opposite-image zeros once)
            y2m = spool.tile([128, G1 * BL], BF16, tag="y2m")
            nc.vector.memset(y2m[:], 0.0)
            # pooled map incl garbage slots (odd-cp slot 5, zero-weighted)
            y2g = spool.tile([128, G1], BF16, tag="y2")
            nc.vector.memset(y2g[:], 1.0)

            rp_tiles = [
                spool.tile([128, FD], BF16, tag="rp0", name="rp0"),
                spool.tile([128, FD], BF16, tag="rp1", name="rp1"),
            ]
            cur = 0

            def affine(dst, src):
                nc.vector.tensor_scalar(
                    dst, src, 1.0 / PI, 1.1, ALU.mult, ALU.add
                )

            # tap 0 in 3 row-pieces (affine writes the product tile directly)
            p0v = rp_tiles[0][:].rearrange("p (il j) -> p il j", il=RP, j=S)
            for il0, il1 in head:
                utp = wpool.tile([128, il1 - il0, S], BF16, tag="atan")
                nc.scalar.activation(
                    utp[:], xsr[:, il0:il1, 0:S], AFT.Arctan,
                    bias=qst[:, 0:1], scale=wst[:, 0:1],
                )
                affine(p0v[:, il0:il1], utp[:])

            # taps 1..78: full-size stream
            for t in range(1, 79):
                r, c = divmod(t, N)
                xv = xsr[:, r : r + RP, c : c + S]
                ut = wpool.tile([128, RP, S], BF16, tag="atan")
                nc.scalar.activation(
                    ut[:], xv, AFT.Arctan,
                    bias=qst[:, t : t + 1], scale=wst[:, t : t + 1],
                )
                un = wpool.tile([128, FD], BF16, tag="un")
                affine(un[:], ut[:].rearrange("p il j -> p (il j)"))
                nxt = 1 - cur
                nc.vector.tensor_tensor(
                    rp_tiles[nxt][:], rp_tiles[cur][:], un[:], ALU.mult
                )
                cur = nxt

            # tap 79 in j-halves (shrinks the ACT->DVE pipeline skew going
            # into the tail)
            r, c = divmod(79, N)
            nxt = 1 - cur
            nv = rp_tiles[nxt][:].rearrange("p (il j) -> p il j", il=RP, j=S)
            cv = rp_tiles[cur][:].rearrange("p (il j) -> p il j", il=RP, j=S)
            for j0, j1 in [(0, 44), (44, S)]:
                uth = wpool.tile([128, RP, j1 - j0], BF16, tag="atan")
                nc.scalar.activation(
                    uth[:], xsr[:, r : r + RP, c + j0 : c + j1], AFT.Arctan,
                    bias=qst[:, 79:80], scale=wst[:, 79:80],
                )
                unh = wpool.tile([128, RP * (j1 - j0)], BF16, tag="un")
                affine(unh[:], uth[:].rearrange("p il j -> p (il j)"))
                nc.vector.tensor_tensor(
                    nv[:, :, j0:j1], cv[:, :, j0:j1],
                    unh[:].rearrange("p (il j) -> p il j", il=RP, j=j1 - j0),
                    ALU.mult,
                )
            cur = nxt

            # tap 80 in 4 joc-aligned jo-quarters; each quarter's j-pool
            # feeds its own repack DMA immediately (per-joc DMAs overlap
            # the remaining quarters' compute).  Repack: partitions
            # (k, cp) -> (k, joc), free (cp, jos, il) -- this free order
            # keeps both DMA access patterns mergeable to <= 3 dims.
            p1T = spool.tile([128, CP * 6 * RP], BF16, tag="p1T")
            p1j = p1[:].rearrange("p (jo il) -> p jo il", jo=JOP, il=RP)
            p1Td = p1T[:].rearrange(
                "(kk joc) (cp jos il) -> kk joc cp jos il",
                kk=32, joc=CP, jos=6, cp=CP, il=RP,
            )
            r, c = divmod(80, N)
            jq = [(0, 6), (6, 12), (12, 18), (18, 22)]   # jo ranges
            fin = rp_tiles[1 - cur]
            finv = fin[:].rearrange("p (il j) -> p il j", il=RP, j=S)
            curv = rp_tiles[cur][:].rearrange("p (il j) -> p il j", il=RP, j=S)
            finq = fin[:].rearrange(
                "p (il jo jj) -> p il jo jj", il=RP, jo=SP, jj=4
            )
            for qi, (q0, q1) in enumerate(jq):
                j0, j1 = 4 * q0, 4 * q1
                utq = wpool.tile([128, RP, j1 - j0], BF16, tag="atan")
                nc.scalar.activation(
                    utq[:], xsr[:, r : r + RP, c + j0 : c + j1], AFT.Arctan,
                    bias=qst[:, 80:81], scale=wst[:, 80:81],
                )
                unq = wpool.tile([128, RP * (j1 - j0)], BF16, tag="un")
                affine(unq[:], utq[:].rearrange("p il j -> p (il j)"))
                nc.vector.tensor_tensor(
                    finv[:, :, j0:j1],
                    curv[:, :, j0:j1],
                    unq[:].rearrange("p (il j) -> p il j", il=RP, j=j1 - j0),
                    ALU.mult,
                )
                nc.vector.tensor_reduce(
                    p1v[:, :, q0:q1],
                    finq[:, :, q0:q1, :],
                    mybir.AxisListType.X,
                    ALU.max,
                )
                # repack DMA for this joc block (jo 6*qi..6*qi+6; block 3
                # includes the memset zero-pad columns 22..23)
                nc.sync.dma_start(
                    p1Td[:, qi],
                    p1j[:, 6 * qi : 6 * qi + 6, :],
                )

            # row pool over global row-groups, slot-based: chunk cp owns
            # row-groups io = IO0[cp] + s (6 slots for even cp, 5 + garbage
            # for odd cp); the two groups that cross a chunk boundary
            # (io 5 and 16) are finished with a tiny max against the next
            # chunk's first two rows -- all in free dims, no extra DMA.
            p1Tc = p1T[:].rearrange(
                "p (ce par jos il) -> p ce par jos il",
                ce=2, par=2, jos=6, il=RP,
            )
            y2 = y2g
            y2v = y2[:].rearrange(
                "p (ce par jos s) -> p ce par jos s", ce=2, par=2, jos=6, s=6
            )
            for par in (0, 1):   # chunk parity: il offset 0 (even) / 2 (odd)
                off = OFF[par]
                ein = p1Tc[:, :, par, :, off : off + 4 * 5].rearrange(
                    "p ce jos (s ii) -> p ce jos s ii", s=5, ii=4
                )
                nc.vector.tensor_reduce(
                    y2v[:, :, par, :, 0:5], ein, mybir.AxisListType.X, ALU.max
                )
            # even-cp slot 5: rows il 20:22 ...
            nc.vector.tensor_reduce(
                y2v[:, :, 0, :, 5],
                p1Tc[:, :, 0, :, 20:22],
                mybir.AxisListType.X,
                ALU.max,
            )
            # ... maxed with rows il 0:2 of the following (odd) chunk
            t5 = spool.tile([128, 2 * 6], BF16, tag="t5")
            t5v = t5[:].rearrange("p (ce jos) -> p ce jos", ce=2, jos=6)
            nc.vector.tensor_reduce(
                t5v, p1Tc[:, :, 1, :, 0:2], mybir.AxisListType.X, ALU.max
            )
            nc.vector.tensor_tensor(
                y2v[:, :, 0, :, 5], y2v[:, :, 0, :, 5], t5v, ALU.max
            )

            # ln of the pooled product via the bf16-bits trick, folded into
            # the scatter to the image-masked two-column fc1 rhs.  Slots
            # 0..4 don't depend on the crossing-group fix above, so they
            # scatter first and fc1 (ordered s<5 groups first) can start
            # while the s=5 slots still finish.
            y2b = y2[:].bitcast(I16).rearrange(
                "p (cp jos s) -> p cp jos s", cp=CP, jos=6, s=6
            )
            y2mv = y2m[:].rearrange("p (g c) -> p g c", g=G1, c=BL)
            y2mc = y2m[:].rearrange(
                "p (cp jos s c) -> p cp jos s c", cp=CP, jos=6, s=6, c=BL
            )
            for p0, col in ((0, 0), (64, 1)):
                nc.vector.tensor_scalar(
                    y2mc[p0 : p0 + 64, :, :, 0:5, col],
                    y2b[p0 : p0 + 64, :, :, 0:5],
                    LN_S0, LN_S1, ALU.mult, ALU.add,
                )
            for p0, col in ((0, 0), (64, 1)):
                nc.vector.tensor_scalar(
                    y2mc[p0 : p0 + 64, :, :, 5, col],
                    y2b[p0 : p0 + 64, :, :, 5],
                    LN_S0, LN_S1, ALU.mult, ALU.add,
                )

            # fc1: accumulating K=128 matmuls, both images per matmul;
            # groups whose weights are all-zero (odd-cp garbage slot 5)
            # are skipped outright
            glist = [g for g in range(G1) if g % 6 != 5] + [
                g for g in range(G1) if g % 6 == 5 and (g // 36) % 2 == 0
            ]
            ph = ppool.tile([128, BL], F32, tag="ph")
            for gi, g in enumerate(glist):
                nc.tensor.matmul(
                    ph[:],
                    w1t[:, g * 128 : (g + 1) * 128],
                    y2mv[:, g],
                    start=(gi == 0),
                    stop=(gi == len(glist) - 1),
                )
            # relu(ph + b1) on DVE (keeps ACT free of table traffic)
            h = spool.tile([128, BL], F32, tag="h")
            nc.vector.tensor_scalar(
                h[:], ph[:], w2bt[:, 10:11], 0.0, ALU.add, ALU.max
            )

            # fc2 + bias
            po = ppool.tile([10, BL], F32, tag="po")
            nc.tensor.matmul(
                po[:], w2bt[:, 0:10], h[:], start=True, stop=True
            )
            osb = spool.tile([10, BL], F32, tag="osb")
            nc.vector.tensor_scalar(
                osb[:], po[:], b2t[:], 0.0, ALU.add, ALU.add
            )
            nc.sync.dma_start(out[:], osb[:])

    nc.compile()
    return nc


def _prep_inputs(x, w, q, fc1_w, fc1_b, fc2_w, fc2_b):
    x = np.asarray(x, np.float32)
    w = np.asarray(w, np.float32)
    q = np.asarray(q, np.float32)
    fc1_w = np.asarray(fc1_w, np.float32)
    fc1_b = np.asarray(fc1_b, np.float32)
    fc2_w = np.asarray(fc2_w, np.float32)
    fc2_b = np.asarray(fc2_b, np.float32)

    # halo chunks: [B, CP, HALO, IMG]; 22*3 + 30 = 96 exactly, no padding
    xh = np.stack(
        [x[:, RP * cp : RP * cp + HALO, :] for cp in range(CP)], axis=1
    )

    # ws/qs rows follow p = 4*(b*16+m) + cp
    wm = np.repeat(10.0 * w.reshape(M, 81), CP, axis=0)      # [64, 81]
    qm = np.repeat(-10.0 * q.reshape(M, 81), CP, axis=0)
    wq = np.hstack([np.tile(wm, (BL, 1)), np.tile(qm, (BL, 1))])  # [128,162]
    wq = np.ascontiguousarray(wq)

    # fc1 weights: w1[(k*4+joc) partition, (cp, jos, s) group, n]
    #   = fc1_w[n, io*352 + jo*16 + m], jo = 6*joc + jos, io = IO0[cp] + s;
    # zero for jo >= 22 and for the garbage slots (odd cp, s == 5);
    # independent of the image half (k = b*16 + m).
    A = fc1_w.reshape(128, SP, SP, M)             # [n, io, jo, m]
    W = np.zeros((M, CP, CP, 6, 6, 128), np.float32)  # [m,joc,cp,jos,s,n]
    for joc in range(CP):
        for jos in range(6):
            jo = 6 * joc + jos
            if jo >= SP:
                continue
            for cp in range(CP):
                for s in range(6):
                    if s == 5 and cp % 2 == 1:
                        continue
                    io = IO0[cp] + s
                    W[:, joc, cp, jos, s, :] = A[:, io, jo, :].T
    half = W.reshape(64, G1 * 128)
    w1 = np.ascontiguousarray(np.tile(half, (BL, 1))).astype(ml_dtypes.bfloat16)

    w2b = np.hstack([fc2_w.T, fc1_b.reshape(128, 1)]).astype(np.float32)
    w2b = np.ascontiguousarray(w2b)
    b2 = fc2_b.reshape(10, 1).astype(np.float32)

    in_maps = []
    for k in range(NCORES):
        arr = xh[BL * k : BL * k + BL]            # [BL, CP, HALO, IMG]
        xpk = np.broadcast_to(arr[:, None], (BL, M, CP, HALO, IMG))
        xpk = np.ascontiguousarray(xpk.reshape(128, HALO * IMG)).astype(
            ml_dtypes.bfloat16
        )
        in_maps.append(dict(xp=xpk, wq=wq, w1=w1, w2b=w2b, b2=b2))
    return in_maps


def kernel(x, w, q, fc1_w, fc1_b, fc2_w, fc2_b):
    if "nc" not in _CACHE:
        _CACHE["nc"] = _build_nc()
    nc = _CACHE["nc"]
    in_maps = _prep_inputs(x, w, q, fc1_w, fc1_b, fc2_w, fc2_b)
    # The axon-tunneled devices occasionally throw a transient
    # NRT_EXEC_UNIT_UNRECOVERABLE on the first execution of a fresh NEFF;
    # a retry has always succeeded with identical results.
    last_err = None
    for attempt in range(3):
        try:
            res = run_bass_kernel_spmd(nc, in_maps, list(range(NCORES)))
            break
        except Exception as e:  # noqa: BLE001 - retry transient device faults
            last_err = e
            import time as _time
            _time.sleep(5 * (attempt + 1))
    else:
        raise last_err
    _CACHE["last_exec_time_ns"] = res.exec_time_ns
    _CACHE["last_results"] = res
    outp = np.empty((B, 10), np.float32)
    for k in range(NCORES):
        o = np.asarray(res.results[k]["out"], np.float32)   # [10, BL]
        outp[BL * k : BL * k + BL, :] = o.T
    return outp


# revision 34
# speedup vs baseline: 1.0014x; 1.0014x over previous
"""Trainium2 Bass kernel for the modified-MDPN dendrite model (v7).

Math per output element (b, i, j, m):
    acc = sum_r log(prod_c u)  with  u = atan(10*(x*w - q))/pi + 1.1
        = log(prod_{r,c} u)    (u > 0 always)
Then 4x4 spatial maxpool, flatten (io, jo, m), fc1(7744->128)+relu,
fc2(128->10).

Device strategy (8 NeuronCores, data parallel over batch, 2 images/core):
  - partitions p = 4*(b*16 + m) + cp (b: 2 images, m: 16 filters, cp: 4
    chunks of 22 output rows; 4*22 = 88 = S exactly, so the atan stream
    carries ZERO padding: free size per tap is 22 rows * 88 cols = 1936
    vs the 2112 of a pool-aligned 12-row chunking).
  - per tap (r, c): one ACT Arctan over [128, 1936] with per-partition
    scale=10w, bias=-10q folded into the activation pre-affine (HW arctan
    is accurate far beyond +-pi/2); one DVE tensor_scalar (u = t/pi + 1.1,
    bf16, 4x mode); one DVE tensor_tensor multiply into a running product
    (bf16, 2x mode, ping-pong).  Cost model: ACT 1798ns/tap (1613 compute
    + 185 SBUF access), DVE 1634ns/tap -- ACT is the critical path and
    runs gapless.  Taps run r-major so the input DMA (x pre-cast to bf16
    on host, row slices in order) stays ahead; tap 0 is split into 3
    row-pieces matched to the first DMA slices.  A dependency-free dummy
    atan at the top pulls the 1283ns Arctan table load to t=0 (it is
    otherwise inserted before the first real atan and inherits its DMA
    waits).
  - ln is monotonic, so the 4x4 maxpool runs on the bf16 *products*:
    col-pool (groups of 4 j) is a free-dim tensor_reduce; the row pool
    crosses the 22-row chunks, so the tiny j-pooled map ([128, 24*22]
    bf16, 2 of 24 jo slots zero-pad) is repacked onto partitions
    p' = 4*(b*16+m) + joc, free (cp, jos, il) -- that free order keeps
    both DMA access patterns mergeable to <= 3 dims, allowing ONE
    SBUF->SBUF DMA per joc block (4 total; each costs ~650ns serialized
    HWDGE time, which is why cp=4 beats cp=8).  The last tap runs in 4
    joc-aligned jo-quarters so each quarter's j-pool launches its repack
    DMA immediately, overlapping the remaining quarters' compute; tap 79
    runs in j-halves to shrink the ACT->DVE skew into the tail.
  - After the repack the row pool is slot-based in free dims: chunk cp
    owns row-groups io = IO0[cp] + s; the two groups crossing a chunk
    boundary are finished with a tiny max against the next chunk's first
    two rows (the next chunk is a free-dim neighbour after the repack).
  - Ln never runs on ACT (avoids Arctan<->Ln table reloads, 1283ns each):
    ln(v) ~= ln2*(bits(v)/128 - 127 + 0.043) via a bf16->int16 bitcast +
    one DVE tensor_scalar, folded into the scatter that builds fc1's rhs.
    ~1e-3 additional relative error on the logits.
  - fc1: contraction over (m, jo, io) runs as 132 accumulating K=128
    matmuls (12 all-zero garbage groups skipped); batch lives on
    partitions, so the rhs is a host-zeroed two-column copy of y2ln
    (col b nonzero only on image-b partitions), giving both images'
    hidden vectors in one PSUM [128, 2].  relu+bias and the fc2 bias run
    on DVE (no ACT table traffic); fc2 is one matmul.

Cost model whole-kernel: 163.1us (v1 12-row-chunk baseline: 175.8us on
the same model).  Breakdown: ~3.9us input-DMA-latency startup, 81-tap
ACT stream ~146us, ~12us tail (DVE pool drain 3.4, last repack DMA
chain 2.8, pool/fc chain 2.9, out-DMA + exit barrier 3.0).
"""

import math
import sys

sys.path.insert(0, "/opt/trn_rl_repo")

import ml_dtypes
import numpy as np

import concourse.bacc as bacc
import concourse.mybir as mybir
from concourse import tile
from concourse.bass_utils import run_bass_kernel_spmd

AFT = mybir.ActivationFunctionType
ALU = mybir.AluOpType
F32 = mybir.dt.float32
BF16 = mybir.dt.bfloat16
I16 = mybir.dt.int16

M = 16          # filters
N = 9           # window side
IMG = 96
S = 88          # sliding-window output side
SP = 22         # pooled side
B = 16          # global batch
NCORES = 8
BL = B // NCORES          # images per core (2)
CP = 4                    # row chunks (22 rows each; 4*22 = 88, no pad)
RP = 22                   # output rows per chunk
HALO = RP + N - 1         # input rows per chunk (30)
FD = RP * S               # free elems per tap instruction (1936)
JOP = 24                  # jo slots incl 2 zero-pads (4 joc * 6 jos)
G1 = 4 * 6 * 6            # fc1 groups (cp, jos, slot) incl garbage slots
IO0 = (0, 6, 11, 17)      # first row-group owned by chunk cp
OFF = (0, 2, 0, 2)        # in-chunk il offset of the first owned group
PI = float(np.pi)
# ln(v) ~= LN_S0 * int16_bits(bf16 v) + LN_S1  (positive normal v)
LN_S0 = math.log(2.0) / 128.0
LN_S1 = math.log(2.0) * (0.043 - 127.0)

_CACHE = {}


def _build_nc():
    nc = bacc.Bacc("TRN2", target_bir_lowering=False, debug=False)

    xp = nc.declare_dram_parameter("xp", [128, HALO * IMG], BF16, isOutput=False)
    wq = nc.declare_dram_parameter("wq", [128, 162], F32, isOutput=False)
    w1 = nc.declare_dram_parameter("w1", [128, G1 * 128], BF16, isOutput=False)
    w2b = nc.declare_dram_parameter("w2b", [128, 11], F32, isOutput=False)
    b2 = nc.declare_dram_parameter("b2", [10, 1], F32, isOutput=False)
    out = nc.declare_dram_parameter("out", [10, BL], F32, isOutput=True)

    with tile.TileContext(nc) as tc:
        with (
            tc.tile_pool(name="consts", bufs=1) as cpool,
            tc.tile_pool(name="work", bufs=6) as wpool,
            tc.tile_pool(name="state", bufs=1) as spool,
            tc.tile_pool(name="psum", bufs=1, space="PSUM") as ppool,
        ):
            xs = cpool.tile([128, HALO * IMG], BF16, tag="xs")
            wqt = cpool.tile([128, 162], F32, tag="wqt")
            w1t = cpool.tile([128, G1 * 128], BF16, tag="w1t")
            w2bt = cpool.tile([128, 11], F32, tag="w2bt")
            b2t = cpool.tile([10, 1], F32, tag="b2t")
            wst = wqt[:, 0:81]
            qst = wqt[:, 81:162]

            xsr = xs[:].rearrange("p (il j) -> p il j", il=HALO, j=IMG)
            xpr = xp.rearrange("p (il j) -> p il j", il=HALO, j=IMG)

            # Table-load warmup: the Arctan table load is inserted right
            # before the first ACT instruction and inherits its DMA waits,
            # so give ACT a dependency-free dummy atan first -- the 1283ns
            # load then runs at t=0 under the input-DMA latency.  All DMAs
            # stay on the SP queue (an ACT-queue DMA triggers an extra
            # set-0 table load).  First x slice leads, then ws/qs, the
            # remaining slices, and the big fc1 weights last.
            scr = cpool.tile([128, 1], F32, tag="scr")
            nc.vector.memset(scr[:], 0.0)
            nc.scalar.activation(scr[:], scr[:], AFT.Arctan)
            head = [(0, 11), (11, RP)]
            nc.sync.dma_start(xsr[:, 0:11], xpr[:, 0:11])
            nc.sync.dma_start(wqt[:], wq[:])
            for il0, il1 in head[1:]:
                nc.sync.dma_start(xsr[:, il0:il1], xpr[:, il0:il1])
            nc.sync.dma_start(xsr[:, RP:HALO], xpr[:, RP:HALO])
            nc.sync.dma_start(b2t[:], b2[:])
            nc.sync.dma_start(w2bt[:], w2b[:])
            nc.sync.dma_start(w1t[:], w1[:])

            # j-pooled map, layout (jo, il); jo slots 22..23 stay zero.
            p1 = spool.tile([128, JOP * RP], BF16, tag="p1")
            nc.vector.memset(p1[:], 0.0)
            p1v = p1[:].rearrange("p (jo il) -> p il jo", jo=JOP, il=RP)
            # image-masked two-column rhs for fc1 (memset covers the
            # opposite-image zeros once)
            y2m = spool.tile([128, G1 * BL], BF16, tag="y2m")
            nc.vector.memset(y2m[:], 0.0)
            # pooled map incl garbage slots (odd-cp slot 5, zero-weighted)
            y2g = spool.tile([128, G1], BF16, tag="y2")
            nc.vector.memset(y2g[:], 1.0)

            rp_tiles = [
                spool.tile([128, FD], BF16, tag="rp0", name="rp0"),
                spool.tile([128, FD], BF16, tag="rp1", name="rp1"),
            ]
            cur = 0

            def affine(dst, src):
                nc.vector.tensor_scalar(
                    dst, src, 1.0 / PI, 1.1, ALU.mult, ALU.add
                )

            # tap 0 in 3 row-pieces (affine writes the product tile directly)
            p0v = rp_tiles[0][:].rearrange("p (il j) -> p il j", il=RP, j=S)
            for il0, il1 in head:
                utp = wpool.tile([128, il1 - il0, S], BF16, tag="atan")
                nc.scalar.activation(
                    utp[:], xsr[:, il0:il1, 0:S], AFT.Arctan,
                    bias=qst[:, 0:1], scale=wst[:, 0:1],
                )
                affine(p0v[:, il0:il1], utp[:])

            # taps 1..78: full-size stream
            for t in range(1, 79):
                r, c = divmod(t, N)
                xv = xsr[:, r : r + RP, c : c + S]
                ut = wpool.tile([128, RP, S], BF16, tag="atan")
                nc.scalar.activation(
                    ut[:], xv, AFT.Arctan,
                    bias=qst[:, t : t + 1], scale=wst[:, t : t + 1],
                )
                un = wpool.tile([128, FD], BF16, tag="un")
                affine(un[:], ut[:].rearrange("p il j -> p (il j)"))
                nxt = 1 - cur
                nc.vector.tensor_tensor(
                    rp_tiles[nxt][:], rp_tiles[cur][:], un[:], ALU.mult
                )
                cur = nxt

            # tap 79 in j-halves (shrinks the ACT->DVE pipeline skew going
            # into the tail)
            r, c = divmod(79, N)
            nxt = 1 - cur
            nv = rp_tiles[nxt][:].rearrange("p (il j) -> p il j", il=RP, j=S)
            cv = rp_tiles[cur][:].rearrange("p (il j) -> p il j", il=RP, j=S)
            for j0, j1 in [(0, 44), (44, S)]:
                uth = wpool.tile([128, RP, j1 - j0], BF16, tag="atan")
                nc.scalar.activation(
                    uth[:], xsr[:, r : r + RP, c + j0 : c + j1], AFT.Arctan,
                    bias=qst[:, 79:80], scale=wst[:, 79:80],
                )
                unh = wpool.tile([128, RP * (j1 - j0)], BF16, tag="un")
                affine(unh[:], uth[:].rearrange("p il j -> p (il j)"))
                nc.vector.tensor_tensor(
                    nv[:, :, j0:j1], cv[:, :, j0:j1],
                    unh[:].rearrange("p (il j) -> p il j", il=RP, j=j1 - j0),
                    ALU.mult,
                )
            cur = nxt

            # tap 80 in 4 joc-aligned jo-quarters; each quarter's j-pool
            # feeds its own repack DMA immediately (per-joc DMAs overlap
            # the remaining quarters' compute).  Repack: partitions
            # (k, cp) -> (k, joc), free (cp, jos, il) -- this free order
            # keeps both DMA access patterns mergeable to <= 3 dims.
            p1T = spool.tile([128, CP * 6 * RP], BF16, tag="p1T")
            p1j = p1[:].rearrange("p (jo il) -> p jo il", jo=JOP, il=RP)
            p1Td = p1T[:].rearrange(
                "(kk joc) (cp jos il) -> kk joc cp jos il",
                kk=32, joc=CP, jos=6, cp=CP, il=RP,
            )
            r, c = divmod(80, N)
            jq = [(0, 6), (6, 12), (12, 18), (18, 22)]   # jo ranges
            fin = rp_tiles[1 - cur]
            finv = fin[:].rearrange("p (il j) -> p il j", il=RP, j=S)
            curv = rp_tiles[cur][:].rearrange("p (il j) -> p il j", il=RP, j=S)
            finq = fin[:].rearrange(
                "p (il jo jj) -> p il jo jj", il=RP, jo=SP, jj=4
            )
            for qi, (q0, q1) in enumerate(jq):
                j0, j1 = 4 * q0, 4 * q1
                utq = wpool.tile([128, RP, j1 - j0], BF16, tag="atan")
                nc.scalar.activation(
                    utq[:], xsr[:, r : r + RP, c + j0 : c + j1], AFT.Arctan,
                    bias=qst[:, 80:81], scale=wst[:, 80:81],
                )
                unq = wpool.tile([128, RP * (j1 - j0)], BF16, tag="un")
                affine(unq[:], utq[:].rearrange("p il j -> p (il j)"))
                nc.vector.tensor_tensor(
                    finv[:, :, j0:j1],
                    curv[:, :, j0:j1],
                    unq[:].rearrange("p (il j) -> p il j", il=RP, j=j1 - j0),
                    ALU.mult,
                )
                nc.vector.tensor_reduce(
                    p1v[:, :, q0:q1],
                    finq[:, :, q0:q1, :],
                    mybir.AxisListType.X,
                    ALU.max,
                )
                # repack DMA for this joc block (jo 6*qi..6*qi+6; block 3
                # includes the memset zero-pad columns 22..23)
                nc.sync.dma_start(
                    p1Td[:, qi],
                    p1j[:, 6 * qi : 6 * qi + 6, :],
                )

            # row pool over global row-groups, slot-based: chunk cp owns
            # row-groups io = IO0[cp] + s (6 slots for even cp, 5 + garbage
            # for odd cp); the two groups that cross a chunk boundary
            # (io 5 and 16) are finished with a tiny max against the next
            # chunk's first two rows -- all in free dims, no extra DMA.
            p1Tc = p1T[:].rearrange(
                "p (ce par jos il) -> p ce par jos il",
                ce=2, par=2, jos=6, il=RP,
            )
            y2 = y2g
            y2v = y2[:].rearrange(
                "p (ce par jos s) -> p ce par jos s", ce=2, par=2, jos=6, s=6
            )
            for par in (0, 1):   # chunk parity: il offset 0 (even) / 2 (odd)
                off = OFF[par]
                ein = p1Tc[:, :, par, :, off : off + 4 * 5].rearrange(
                    "p ce jos (s ii) -> p ce jos s ii", s=5, ii=4
                )
                nc.vector.tensor_reduce(
                    y2v[:, :, par, :, 0:5], ein, mybir.AxisListType.X, ALU.max
                )
            # even-cp slot 5: rows il 20:22 ...
            nc.vector.tensor_reduce(
                y2v[:, :, 0, :, 5],
                p1Tc[:, :, 0, :, 20:22],
                mybir.AxisListType.X,
                ALU.max,
            )
            # ... maxed with rows il 0:2 of the following (odd) chunk
            t5 = spool.tile([128, 2 * 6], BF16, tag="t5")
            t5v = t5[:].rearrange("p (ce jos) -> p ce jos", ce=2, jos=6)
            nc.vector.tensor_reduce(
                t5v, p1Tc[:, :, 1, :, 0:2], mybir.AxisListType.X, ALU.max
            )
            nc.vector.tensor_tensor(
                y2v[:, :, 0, :, 5], y2v[:, :, 0, :, 5], t5v, ALU.max
            )

            # ln of the pooled product via the bf16-bits trick, folded into
            # the scatter to the image-masked two-column fc1 rhs.  Slots
            # 0..4 don't depend on the crossing-group fix above, so they
            # scatter first and fc1 (ordered s<5 groups first) can start
            # while the s=5 slots still finish.
            y2b = y2[:].bitcast(I16).rearrange(
                "p (cp jos s) -> p cp jos s", cp=CP, jos=6, s=6
            )
            y2mv = y2m[:].rearrange("p (g c) -> p g c", g=G1, c=BL)
            y2mc = y2m[:].rearrange(
                "p (cp jos s c) -> p cp jos s c", cp=CP, jos=6, s=6, c=BL
            )
            for p0, col in ((0, 0), (64, 1)):
                nc.vector.tensor_scalar(
                    y2mc[p0 : p0 + 64, :, :, 0:5, col],
                    y2b[p0 : p0 + 64, :, :, 0:5],
                    LN_S0, LN_S1, ALU.mult, ALU.add,
                )
            for p0, col in ((0, 0), (64, 1)):
                nc.vector.tensor_scalar(
                    y2mc[p0 : p0 + 64, :, :, 5, col],
                    y2b[p0 : p0 + 64, :, :, 5],
                    LN_S0, LN_S1, ALU.mult, ALU.add,
                )

            # fc1: accumulating K=128 matmuls, both images per matmul;
            # groups whose weights are all-zero (odd-cp garbage slot 5)
            # are skipped outright
            glist = [g for g in range(G1) if g % 6 != 5] + [
                g for g in range(G1) if g % 6 == 5 and (g // 36) % 2 == 0
            ]
            ph = ppool.tile([128, BL], F32, tag="ph")
            for gi, g in enumerate(glist):
                nc.tensor.matmul(
                    ph[:],
                    w1t[:, g * 128 : (g + 1) * 128],
                    y2mv[:, g],
                    start=(gi == 0),
                    stop=(gi == len(glist) - 1),
                )
            # relu(ph + b1) on DVE (keeps ACT free of table traffic)
            h = spool.tile([128, BL], F32, tag="h")
            nc.vector.tensor_scalar(
                h[:], ph[:], w2bt[:, 10:11], 0.0, ALU.add, ALU.max
            )

            # fc2 + bias
            po = ppool.tile([10, BL], F32, tag="po")
            nc.tensor.matmul(
                po[:], w2bt[:, 0:10], h[:], start=True, stop=True
            )
            osb = spool.tile([10, BL], F32, tag="osb")
            nc.vector.tensor_scalar(
                osb[:], po[:], b2t[:], 0.0, ALU.add, ALU.add
            )
            nc.sync.dma_start(out[:], osb[:])

    nc.compile()
    return nc


def _prep_inputs(x, w, q, fc1_w, fc1_b, fc2_w, fc2_b):
    x = np.asarray(x, np.float32)
    w = np.asarray(w, np.float32)
    q = np.asarray(q, np.float32)
    fc1_w = np.asarray(fc1_w, np.float32)
    fc1_b = np.asarray(fc1_b, np.float32)
    fc2_w = np.asarray(fc2_w, np.float32)
    fc2_b = np.asarray(fc2_b, np.float32)

    # halo chunks: [B, CP, HALO, IMG]; 22*3 + 30 = 96 exactly, no padding
    xh = np.stack(
        [x[:, RP * cp : RP * cp + HALO, :] for cp in range(CP)], axis=1
    )

    # ws/qs rows follow p = 4*(b*16+m) + cp
    wm = np.repeat(10.0 * w.reshape(M, 81), CP, axis=0)      # [64, 81]
    qm = np.repeat(-10.0 * q.reshape(M, 81), CP, axis=0)
    wq = np.hstack([np.tile(wm, (BL, 1)), np.tile(qm, (BL, 1))])  # [128,162]
    wq = np.ascontiguousarray(wq)

    # fc1 weights: w1[(k*4+joc) partition, (cp, jos, s) group, n]
    #   = fc1_w[n, io*352 + jo*16 + m], jo = 6*joc + jos, io = IO0[cp] + s;
    # zero for jo >= 22 and for the garbage slots (odd cp, s == 5);
    # independent of the image half (k = b*16 + m).
    A = fc1_w.reshape(128, SP, SP, M)             # [n, io, jo, m]
    W = np.zeros((M, CP, CP, 6, 6, 128), np.float32)  # [m,joc,cp,jos,s,n]
    for joc in range(CP):
        for jos in range(6):
            jo = 6 * joc + jos
            if jo >= SP:
                continue
            for cp in range(CP):
                for s in range(6):
                    if s == 5 and cp % 2 == 1:
                        continue
                    io = IO0[cp] + s
                    W[:, joc, cp, jos, s, :] = A[:, io, jo, :].T
    half = W.reshape(64, G1 * 128)
    w1 = np.ascontiguousarray(np.tile(half, (BL, 1))).astype(ml_dtypes.bfloat16)

    w2b = np.hstack([fc2_w.T, fc1_b.reshape(128, 1)]).astype(np.float32)
    w2b = np.ascontiguousarray(w2b)
    b2 = fc2_b.reshape(10, 1).astype(np.float32)

    in_maps = []
    for k in range(NCORES):
        arr = xh[BL * k : BL * k + BL]            # [BL, CP, HALO, IMG]
        xpk = np.broadcast_to(arr[:, None], (BL, M, CP, HALO, IMG))
        xpk = np.ascontiguousarray(xpk.reshape(128, HALO * IMG)).astype(
            ml_dtypes.bfloat16
        )
        in_maps.append(dict(xp=xpk, wq=wq, w1=w1, w2b=w2b, b2=b2))
    return in_maps


def kernel(x, w, q, fc1_w, fc1_b, fc2_w, fc2_b):
    if "nc" not in _CACHE:
        _CACHE["nc"] = _build_nc()
    nc = _CACHE["nc"]
    in_maps = _prep_inputs(x, w, q, fc1_w, fc1_b, fc2_w, fc2_b)
    # The axon-tunneled devices occasionally throw a transient
    # NRT_EXEC_UNIT_UNRECOVERABLE on the first execution of a fresh NEFF;
    # a retry has always succeeded with identical results.
    last_err = None
    for attempt in range(3):
        try:
            res = run_bass_kernel_spmd(nc, in_maps, list(range(NCORES)))
            break
        except Exception as e:  # noqa: BLE001 - retry transient device faults
            last_err = e
            import time as _time
            _time.sleep(5 * (attempt + 1))
    else:
        raise last_err
    _CACHE["last_exec_time_ns"] = res.exec_time_ns
    _CACHE["last_results"] = res
    outp = np.empty((B, 10), np.float32)
    for k in range(NCORES):
        o = np.asarray(res.results[k]["out"], np.float32)   # [10, BL]
        outp[BL * k : BL * k + BL, :] = o.T
    return outp
